# revision 29
# baseline (speedup 1.0000x reference)
"""Bahdanau attention Trainium2 kernel.

Problem: B=32, S=4096, D=256, UNITS=256 (f32)
  q_proj = query @ W2                         [B, U]
  v_proj = values @ W1                        [B, S, U]
  score  = tanh(v_proj + q_proj[:,None]) @ V  [B, S, 1]
  wts    = softmax(score, axis=1)             [B, S]
  ctx    = sum(wts * values, axis=1)          [B, D]
returns (ctx, wts)

Sharding: data-parallel over batch, 4 batches per core on 8 cores.

Device-side orientation is fully transposed (d / u on partitions):
  vt[k]   = values[b].T d-block k            [128, 4096]   (host pre-transposed)
  v_projT = W1[k,m].T @ vt[k]  (accum k)     [128u, s]     fp32r matmuls
  tanhT   = tanh(v_projT + q_projT[u])       ACT, bias = per-partition q_proj
  score   = Vrep[m].T @ tanhT[m] (accum m)   [128, s]  -- V replicated across
            128 lhsT columns so every PSUM partition gets the same score row
            (free broadcast: same cycles as M=1)
  ew      = exp(score)  (no max-subtract: |score| <= sum|V| ~ 13, safe in f32)
  denom   = per-chunk DVE tensor_scalar accum (2x_2P mode; ACT accum_out
            costs an extra ACT_READ_ACCUMULATOR step per op, Pool lacks the
            accum opcode entirely)
  ctxT[k] = reduce_s(vt[k] * ew) / denom     custom-DVE TENSOR_TENSOR_REDUCE,
            chunk-chained through its accumulator-init scalar
  wts row = ew[0] / denom                    split GPSIMD/DVE, one DMA per row
"""

import numpy as np

B, S, D, U = 32, 4096, 256, 256
NCORES = 8
BPC = B // NCORES  # batches per core
KB = D // 128      # d partition-blocks
MB = U // 128      # u partition-blocks
NC = 512           # matmul free-dim chunk
NJ = S // NC

_CACHE = {}


def _build():
    import concourse.bass as bass
    import concourse.mybir as mybir
    import concourse.tile as tile
    from concourse import bacc
    from concourse.bass import ts
    from concourse.dve_ops import TENSOR_TENSOR_REDUCE

    F32 = mybir.dt.float32
    F32R = mybir.dt.float32r
    AF = mybir.ActivationFunctionType
    ALU = mybir.AluOpType

    nc = bacc.Bacc()
    CB = KB * MB * 128 * 2 + MB * 128 + KB * BPC  # blob columns
    vt_d = nc.declare_dram_parameter("vt", [BPC, KB, 128, S], F32R, isOutput=False)
    blob_d = nc.declare_dram_parameter("blob", [128, CB], F32R, isOutput=False)
    ctx_d = nc.declare_dram_parameter("ctx_out", [128, BPC, KB], F32, isOutput=True)
    w_d = nc.declare_dram_parameter("w_out", [BPC, S], F32, isOutput=True)

    with tile.TileContext(nc) as tc:
        with (
            tc.tile_pool(name="consts", bufs=1) as consts,
            tc.tile_pool(name="vt", bufs=2) as vtp,
            tc.tile_pool(name="ew", bufs=2) as ewp,
            tc.tile_pool(name="tt", bufs=3) as ttp,
            tc.tile_pool(name="scr", bufs=2) as scrp,
            tc.tile_pool(name="small", bufs=2) as smallp,
            tc.tile_pool(name="ps", bufs=2, space="PSUM") as psp,
            tc.tile_pool(name="psq", bufs=1, space="PSUM") as psqp,
        ):
            # ---- constants: one blob, one DMA, one semaphore ----
            blob_sb = consts.tile([128, CB], F32R, tag="blob", name="blob")
            nc.sync.dma_start(blob_sb[:], blob_d[:])
            nblk = KB * MB
            w1_sb = [[blob_sb[:, (k * MB + m) * 128:(k * MB + m + 1) * 128]
                      for m in range(MB)] for k in range(KB)]
            w2_sb = [[blob_sb[:, (nblk + k * MB + m) * 128:(nblk + k * MB + m + 1) * 128]
                      for m in range(MB)] for k in range(KB)]
            vrep_sb = [blob_sb[:, (2 * nblk + m) * 128:(2 * nblk + m + 1) * 128]
                       for m in range(MB)]
            q0 = (2 * nblk + MB) * 128
            qT_sb = [blob_sb[:, q0 + k * BPC: q0 + (k + 1) * BPC]
                     for k in range(KB)]

            # ---- q_projT = W2.T @ queryT : [128u, BPC] per u-block ----
            qp_ps = psqp.tile([128, MB * BPC], F32, tag="qp")
            for m in range(MB):
                for k in range(KB):
                    nc.tensor.matmul(
                        qp_ps[:, ts(m, BPC)], w2_sb[k][m], qT_sb[k],
                        start=(k == 0), stop=(k == KB - 1))
            qp_sb = consts.tile([128, MB * BPC], F32, tag="qp_sb")
            nc.scalar.copy(qp_sb[:], qp_ps[:])

            ctx_all = consts.tile([128, BPC, KB], F32, tag="ctx_all")

            # ---- per-batch pipeline ----
            for b in range(BPC):
                vt_sb = [vtp.tile([128, S], F32R, tag=f"vt{k}", name=f"vt{k}") for k in range(KB)]
                nh = 8 if b == 0 else 2
                H = S // nh
                for h in range(nh):
                    for k in range(KB):
                        nc.sync.dma_start(vt_sb[k][:, h * H:(h + 1) * H],
                                          vt_d[b, k, :, h * H:(h + 1) * H])

                ew = ewp.tile([128, S], F32, tag="ew")
                dacc = smallp.tile([128, NJ], F32, tag="dacc")
                cacc = smallp.tile([128, KB], F32, tag="cacc")
                for j in range(NJ):
                    pv = [psp.tile([128, NC], F32, tag=f"pv{m}", name=f"pv{m}") for m in range(MB)]
                    for m in range(MB):
                        for k in range(KB):
                            nc.tensor.matmul(
                                pv[m][:],
                                w1_sb[k][m],
                                vt_sb[k][:, ts(j, NC)],
                                start=(k == 0), stop=(k == KB - 1))
                    tt = [ttp.tile([128, NC], F32R, tag=f"tt{m}", name=f"tt{m}") for m in range(MB)]
                    for m in range(MB):
                        nc.scalar.activation(
                            tt[m][:], pv[m][:], AF.Tanh,
                            bias=qp_sb[:, m * BPC + b: m * BPC + b + 1])
                    ps = psp.tile([128, NC], F32, tag="ps")
                    for m in range(MB):
                        nc.tensor.matmul(
                            ps[:],
                            vrep_sb[m],
                            tt[m][:],
                            start=(m == 0), stop=(m == MB - 1))
                    nc.scalar.activation(ew[:, ts(j, NC)], ps[:], AF.Exp)
                    gscr = scrp.tile([128, NC], F32, tag="gscr", name="gscr")
                    nc.vector.tensor_scalar(
                        gscr[:], ew[:, ts(j, NC)], 1.0, None,
                        mybir.AluOpType.mult, mybir.AluOpType.add,
                        accum_out=dacc[:, j: j + 1])
                    for k in range(KB):
                        scr = scrp.tile([128, NC], F32, tag="scr", name="scr")
                        nc.vector._custom_dve(
                            TENSOR_TENSOR_REDUCE,
                            out=scr[:], in0=vt_sb[k][:, ts(j, NC)].bitcast(F32),
                            in1=ew[:, ts(j, NC)],
                            s0=(0.0 if j == 0 else cacc[:, k: k + 1]), s1=1.0,
                            accum_out=cacc[:, k: k + 1])

                # denominator + reciprocal (same value on every partition)
                dsum = smallp.tile([128, 1], F32, tag="dsum")
                nc.vector.tensor_reduce(dsum[:], dacc[:], axis=mybir.AxisListType.X,
                                        op=ALU.add)
                rinv = smallp.tile([128, 1], F32, tag="rinv")
                nc.vector.reciprocal_approx_fast(rinv[:], dsum[:])

                nc.vector.tensor_scalar_mul(ctx_all[:, b, :], cacc[:], rinv[:])

                # weights row: normalize row 0 in place (split DVE/Pool), DMA out
                Hw = 1024 if b == BPC - 1 else 2048
                nc.gpsimd.tensor_scalar_mul(ew[0:1, :Hw], ew[0:1, :Hw],
                                            rinv[0:1, :])
                nc.vector.tensor_scalar_mul(ew[0:1, Hw:], ew[0:1, Hw:],
                                            rinv[0:1, :])
                nc.sync.dma_start(w_d[b: b + 1, :], ew[0:1, :])

            nc.sync.dma_start(ctx_d[:], ctx_all[:])
    nc.compile()
    return nc


def _get_nc():
    if "nc" not in _CACHE:
        _CACHE["nc"] = _build()
    return _CACHE["nc"]


LAST_RESULT = None


def kernel(query, values, W1, W2, V):
    global LAST_RESULT
    from concourse.bass_utils import run_bass_kernel_spmd

    query = np.ascontiguousarray(np.asarray(query, dtype=np.float32))
    values = np.ascontiguousarray(np.asarray(values, dtype=np.float32))
    W1 = np.ascontiguousarray(np.asarray(W1, dtype=np.float32))
    W2 = np.ascontiguousarray(np.asarray(W2, dtype=np.float32))
    V = np.ascontiguousarray(np.asarray(V, dtype=np.float32))

    vt = np.ascontiguousarray(values.transpose(0, 2, 1)).reshape(B, KB, 128, S)
    qT = np.ascontiguousarray(query.T).reshape(KB, 128, B)
    w1s = W1.reshape(KB, 128, MB, 128)
    w2s = W2.reshape(KB, 128, MB, 128)
    vrep = np.broadcast_to(V.reshape(MB, 128, 1), (MB, 128, 128))

    in_maps = []
    for c in range(NCORES):
        bs = slice(c * BPC, (c + 1) * BPC)
        blob = np.concatenate(
            [w1s[k, :, m, :] for k in range(KB) for m in range(MB)]
            + [w2s[k, :, m, :] for k in range(KB) for m in range(MB)]
            + [vrep[m] for m in range(MB)]
            + [qT[k][:, bs] for k in range(KB)], axis=1)
        in_maps.append({
            "vt": np.ascontiguousarray(vt[bs]),
            "blob": np.ascontiguousarray(blob),
        })

    res = run_bass_kernel_spmd(_get_nc(), in_maps, list(range(NCORES)))
    LAST_RESULT = res

    weights = np.concatenate([res.results[c]["w_out"] for c in range(NCORES)], 0)
    ctx_raw = np.stack([res.results[c]["ctx_out"] for c in range(NCORES)])
    # [core, p, b, k] -> [core, b, k, p] -> [B, D]
    context = ctx_raw.transpose(0, 2, 3, 1).reshape(B, D)
    return context, weights
